# revision 20
# baseline (speedup 1.0000x reference)
"""Trainium2 Bass kernel for nn_CrossAttentionCLSHead.

B=8, L=2048, D=128, H=1024, NH=16, HD=64. Data-parallel: one batch per core.

Per-core pipeline:
  P1: LN(desc) -> qn (q-norm affine + qmask), qT (PE transpose, bf16),
      qp = qT.T @ Wq' (bf16 matmul, f32 psum; 1/8 scale + bias in evac),
      qpT (PE transpose, f32r).
  P2: per k-tile: LN(dna) (kv-norm folded into weights on host) -> z (bf16),
      PE-transpose -> kvT[h,k] (bf16); vp = kvT.T @ Wv' + bv (bf16).
  P3: per head-pair t: kpT[:, t-cols] = Wk'.T @ kvT (+bk, evac f32r);
      per head: scores = qpT.T @ kpT (f32r, K=64), exp (ACT, f32),
      mask-mult + row-sum S (DVE stt w/ accum), attn = mexp * (1/S) -> f32 DMA
      out + bf16 copy (gpsimd); PE-transpose attn_bf -> expT (bf16);
      ctx[hd,q] = vp.T @ expT (bf16, head-major base-0 psum);
      out_proj accumulation for this head (K=64, head-major WoT, bf16).
  P4: res = op_psum + (bo + qn), out-LN, pooled = qmask_pool.T @ ln_out (f32r);
      out-norm affine applied on host.
"""

import sys

if "/opt/trn_rl_repo" not in sys.path:
    sys.path.insert(0, "/opt/trn_rl_repo")

import numpy as np

import concourse.bacc as bacc
import concourse.bass as bass
import concourse.tile as tile
from concourse import mybir
from concourse.bass_utils import run_bass_kernel_spmd

B, L, D, H, NH = 8, 2048, 128, 1024, 16
HD = H // NH          # 64
KT = L // 128         # 16 k-tiles
HC = H // 128         # 8 h-chunks
NP = NH // 2          # 8 head pairs
EPS = 1e-5
SCALE = 1.0 / np.sqrt(HD)

F32 = mybir.dt.float32
F32R = mybir.dt.float32r
BF16 = mybir.dt.bfloat16

_CACHE = {}


def _build_program():
    nc = bacc.Bacc("TRN2", target_bir_lowering=False, debug=False, num_devices=B)

    # ---- DRAM I/O (per core) ----
    d_desc = nc.dram_tensor("desc", [D, H], F32, kind="ExternalInput")
    d_dna = nc.dram_tensor("dna", [L, H], F32, kind="ExternalInput")
    d_wq = nc.dram_tensor("wq", [128, HC, H], BF16, kind="ExternalInput")
    d_wv = nc.dram_tensor("wv", [128, HC, H], BF16, kind="ExternalInput")
    d_wk = nc.dram_tensor("wk", [128, NP, HC, 128], BF16, kind="ExternalInput")
    d_wo = nc.dram_tensor("wo", [64, NH, H], BF16, kind="ExternalInput")
    d_bq = nc.dram_tensor("bq", [H], F32, kind="ExternalInput")  # pre-scaled
    d_bkt = nc.dram_tensor("bkt", [128, HC], F32, kind="ExternalInput")
    d_bv = nc.dram_tensor("bv", [H], F32, kind="ExternalInput")
    d_bo = nc.dram_tensor("bo", [H], F32, kind="ExternalInput")
    d_wqn = nc.dram_tensor("wqn", [H], F32, kind="ExternalInput")
    d_bqn = nc.dram_tensor("bqn", [H], F32, kind="ExternalInput")
    d_km = nc.dram_tensor("kmask", [L], BF16, kind="ExternalInput")
    d_qm = nc.dram_tensor("qmask", [D, 1], F32, kind="ExternalInput")
    d_qmp = nc.dram_tensor("qmask_pool", [D, 1], F32R, kind="ExternalInput")
    d_idf = nc.dram_tensor("ident_f32", [128, 128], F32, kind="ExternalInput")
    d_idb = nc.dram_tensor("ident_bf16", [128, 128], BF16, kind="ExternalInput")
    d_attn = nc.dram_tensor("attn", [NH, D, L], F32, kind="ExternalOutput")
    d_pooled = nc.dram_tensor("pooled", [1, H], F32, kind="ExternalOutput")

    def ln_stats(tc_pool, x, n_free):
        """(rstd, neg_mu_rstd) [128,1] f32 for LN over the free dim."""
        nsub = n_free // 512
        stats = tc_pool.tile([128, nsub, 6], F32, tag="ln_stats")
        for s in range(nsub):
            nc.vector.bn_stats(out=stats[:, s, :], in_=x[:, s * 512:(s + 1) * 512])
        mv = tc_pool.tile([128, 2], F32, tag="ln_mv")
        nc.vector.bn_aggr(out=mv[:], in_=stats[:])
        rstd = tc_pool.tile([128, 1], F32, tag="ln_rstd")
        nc.scalar.activation(out=rstd[:], in_=mv[:, 1:2],
                             func=mybir.ActivationFunctionType.Sqrt, bias=eps_t[:])
        nc.vector.reciprocal(out=rstd[:], in_=rstd[:])
        nmr = tc_pool.tile([128, 1], F32, tag="ln_nmr")
        nc.vector.scalar_tensor_tensor(out=nmr[:], in0=mv[:, 0:1], scalar=-1.0,
                                       in1=rstd[:], op0=mybir.AluOpType.mult,
                                       op1=mybir.AluOpType.mult)
        return rstd, nmr

    with tile.TileContext(nc) as tc:
        from contextlib import ExitStack
        es = ExitStack()
        with es:
            singles = es.enter_context(tc.tile_pool(name="singles", bufs=1))
            persist = es.enter_context(tc.tile_pool(name="persist", bufs=1))
            lnp = es.enter_context(tc.tile_pool(name="lnp", bufs=3))
            wkp = es.enter_context(tc.tile_pool(name="wkp", bufs=2))

            p2w_cm = tc.tile_pool(name="p2w", bufs=1)
            p2w = p2w_cm.__enter__()
            xkp_cm = tc.tile_pool(name="xkp", bufs=4)
            xkp = xkp_cm.__enter__()
            p1_cm = tc.tile_pool(name="p1", bufs=1)
            p1 = p1_cm.__enter__()

            # ---- DMA priority order: q-path first, dna interleaved ----
            xq = p1.tile([128, H], F32)
            nc.sync.dma_start(out=xq[:], in_=d_desc[:])
            wqn_r = p1.tile([1, H], F32)
            nc.sync.dma_start(out=wqn_r[:], in_=d_wqn.ap().unsqueeze(0))
            bqn_r = p1.tile([1, H], F32)
            nc.sync.dma_start(out=bqn_r[:], in_=d_bqn.ap().unsqueeze(0))
            bv_r = singles.tile([1, H], F32)
            nc.sync.dma_start(out=bv_r[:], in_=d_bv.ap().unsqueeze(0))
            bq_r = p1.tile([1, H], F32)
            nc.sync.dma_start(out=bq_r[:], in_=d_bq.ap().unsqueeze(0))
            bo_r = p1.tile([1, H], F32)
            nc.sync.dma_start(out=bo_r[:], in_=d_bo.ap().unsqueeze(0))
            qm_t = singles.tile([128, 1], F32)
            nc.sync.dma_start(out=qm_t[:], in_=d_qm[:])
            bkt_t = singles.tile([128, HC], F32)
            nc.sync.dma_start(out=bkt_t[:], in_=d_bkt[:])
            km_r = p1.tile([1, L], BF16)
            nc.sync.dma_start(out=km_r[:], in_=d_km.ap().unsqueeze(0))
            ident = singles.tile([128, 128], F32)
            nc.sync.dma_start(out=ident[:], in_=d_idf[:])
            identb = singles.tile([128, 128], BF16)
            nc.sync.dma_start(out=identb[:], in_=d_idb[:])
            wq_t = p1.tile([128, HC, H], BF16)
            nc.sync.dma_start(out=wq_t[:, 0:4, :], in_=d_wq[:, 0:4, :])
            xk_pre = {}
            for i in range(2):
                xk = xkp.tile([128, H], F32, tag="xk")
                nc.sync.dma_start(out=xk[:], in_=d_dna[i * 128:(i + 1) * 128, :])
                xk_pre[i] = xk
            wv_t = p2w.tile([128, HC, H], BF16)
            nc.sync.dma_start(out=wv_t[:, 0:4, :], in_=d_wv[:, 0:4, :])
            nc.sync.dma_start(out=wq_t[:, 4:8, :], in_=d_wq[:, 4:8, :])
            xk = xkp.tile([128, H], F32, tag="xk")
            nc.sync.dma_start(out=xk[:], in_=d_dna[2 * 128:3 * 128, :])
            xk_pre[2] = xk
            nc.sync.dma_start(out=wv_t[:, 4:8, :], in_=d_wv[:, 4:8, :])
            wk0 = wkp.tile([128, HC, 128], BF16, tag="wk")
            nc.sync.dma_start(out=wk0[:], in_=d_wk[:, 0, :, :])

            # ---- constants / broadcasts on idle gpsimd ----
            eps_t = singles.tile([128, 1], F32)
            nc.vector.memset(eps_t[:], EPS)
            wqn_t = p1.tile([128, H], F32)
            nc.gpsimd.partition_broadcast(wqn_t[:], wqn_r[:])
            bqn_t = p1.tile([128, H], F32)
            nc.gpsimd.partition_broadcast(bqn_t[:], bqn_r[:])
            bq_t = p1.tile([128, H], F32)
            nc.gpsimd.partition_broadcast(bq_t[:], bq_r[:])
            maskB = singles.tile([128, L], BF16)
            nc.gpsimd.partition_broadcast(maskB[:], km_r[:])
            qmp_t = singles.tile([128, 1], F32R)

            # ---- persistent intermediates ----
            kvT = persist.tile([128, HC, KT, 128], BF16)   # 32KB/p
            vp = persist.tile([128, KT, H], BF16)          # 32KB/p
            qpT = persist.tile([128, HC, 128], F32R)       # 4KB/p
            qn = persist.tile([128, H], F32)               # 4KB/p (residual)
            resbias = persist.tile([128, H], F32)          # 4KB/p
            ctxT = persist.tile([64, NH, 128], BF16)

            # =========== P1: q path ===========
            with tc.tile_pool(name="p1ps", bufs=2, space="PSUM") as p1ps:
                rstd, nmr = ln_stats(lnp, xq, H)
                zq = p1.tile([128, H], F32)
                nc.scalar.activation(out=zq[:], in_=xq[:],
                                     func=mybir.ActivationFunctionType.Identity,
                                     bias=nmr[:], scale=rstd[:])
                # qn = (z*wqn + bqn) * qmask
                bqnm = p1.tile([128, H], F32)
                nc.vector.tensor_scalar(out=bqnm[:], in0=bqn_t[:], scalar1=qm_t[:],
                                        scalar2=None, op0=mybir.AluOpType.mult)
                t1 = p1.tile([128, H], F32)
                nc.vector.tensor_tensor(out=t1[:], in0=zq[:], in1=wqn_t[:],
                                        op=mybir.AluOpType.mult)
                nc.vector.scalar_tensor_tensor(out=qn[:], in0=t1[:], scalar=qm_t[:],
                                               in1=bqnm[:], op0=mybir.AluOpType.mult,
                                               op1=mybir.AluOpType.add)
                bo_t = p1.tile([128, H], F32)
                nc.gpsimd.partition_broadcast(bo_t[:], bo_r[:])
                nc.vector.tensor_tensor(out=resbias[:], in0=bo_t[:], in1=qn[:],
                                        op=mybir.AluOpType.add)
                # qT (bf16) via PE transpose of qn (f32 input; 8 ops)
                qT = p1.tile([128, HC, 128], BF16)
                for t in range(HC):
                    tp = p1ps.tile([128, 128], F32, tag="qt_ps")
                    nc.tensor.transpose(tp[:], qn[:, t * 128:(t + 1) * 128], ident[:])
                    nc.scalar.copy(out=qT[:, t, :], in_=tp[:])
                # qp = qT.T @ wq (bf16, accumulate over kc) -> [128, H] f32 psum
                qp_ps = p1ps.tile([128, H], F32, tag="qp_ps")
                for kc in range(HC):
                    for nn in range(2):
                        nc.tensor.matmul(qp_ps[:, nn * 512:(nn + 1) * 512],
                                         qT[:, kc, :],
                                         wq_t[:, kc, nn * 512:(nn + 1) * 512],
                                         start=(kc == 0), stop=(kc == HC - 1))
                qp_sb = p1.tile([128, H], F32)
                nc.vector.scalar_tensor_tensor(out=qp_sb[:], in0=qp_ps[:],
                                               scalar=float(SCALE), in1=bq_t[:],
                                               op0=mybir.AluOpType.mult,
                                               op1=mybir.AluOpType.add)
                for t in range(HC):
                    tp = p1ps.tile([128, 128], F32, tag="qt_ps")
                    nc.tensor.transpose(tp[:], qp_sb[:, t * 128:(t + 1) * 128], ident[:])
                    nc.scalar.copy(out=qpT[:, t, :], in_=tp[:])
            p1_cm.__exit__(None, None, None)

            # =========== P2: kv path ===========
            with tc.tile_pool(name="p2", bufs=2) as p2, \
                 tc.tile_pool(name="p2ps", bufs=2, space="PSUM") as p2ps, \
                 tc.tile_pool(name="p2tp", bufs=2, space="PSUM") as p2tp:
                bv_t = p2w.tile([128, H], F32)
                nc.gpsimd.partition_broadcast(bv_t[:], bv_r[:])
                for i in range(KT):
                    if i in xk_pre:
                        xk = xk_pre.pop(i)
                    else:
                        xk = xkp.tile([128, H], F32, tag="xk")
                        nc.sync.dma_start(out=xk[:], in_=d_dna[i * 128:(i + 1) * 128, :])
                    rstd, nmr = ln_stats(lnp, xk, H)
                    zk = p2.tile([128, H], BF16, tag="zk")
                    nc.scalar.activation(out=zk[:], in_=xk[:],
                                         func=mybir.ActivationFunctionType.Identity,
                                         bias=nmr[:], scale=rstd[:])
                    for half in range(2):
                        tp = p2tp.tile([128, 4, 128], BF16, tag="kv_tp")
                        for j in range(4):
                            t = half * 4 + j
                            nc.tensor.transpose(tp[:, j, :],
                                                zk[:, t * 128:(t + 1) * 128], identb[:])
                        nc.scalar.copy(
                            out=kvT[:, half * 4:(half + 1) * 4, i, :], in_=tp[:])
                    vp_ps = p2ps.tile([128, H], F32, tag="vp_ps")
                    for kc in range(HC):
                        for nn in range(2):
                            nc.tensor.matmul(vp_ps[:, nn * 512:(nn + 1) * 512],
                                             kvT[:, kc, i, :],
                                             wv_t[:, kc, nn * 512:(nn + 1) * 512],
                                             start=(kc == 0), stop=(kc == HC - 1))
                    nc.vector.tensor_tensor(out=vp[:, i, :], in0=vp_ps[:], in1=bv_t[:],
                                            op=mybir.AluOpType.add)
            xkp_cm.__exit__(None, None, None)
            p2w_cm.__exit__(None, None, None)

            # =========== P3: head loop (+ out_proj accumulation) ===========
            with tc.tile_pool(name="p3", bufs=2) as p3, \
                 tc.tile_pool(name="p3s", bufs=4) as p3s, \
                 tc.tile_pool(name="wop", bufs=1) as wop, \
                 tc.tile_pool(name="opps", bufs=1, space="PSUM") as opps:
                wo_t = wop.tile([64, NH, H], BF16)
                nc.sync.dma_start(out=wo_t[:], in_=d_wo[:])
                op_ps = opps.tile([128, H], F32)
                loop_ps_cm = [tc.tile_pool(name="kpps", bufs=2, space="PSUM"),
                              tc.tile_pool(name="scps", bufs=2, space="PSUM"),
                              tc.tile_pool(name="tpps", bufs=1, space="PSUM"),
                              tc.tile_pool(name="ctxps", bufs=1, space="PSUM")]
                kpps, scps, tpps, ctxps = [cm.__enter__() for cm in loop_ps_cm]
                for t in range(NP):
                    if t == 0:
                        wk_t = wk0
                    else:
                        wk_t = wkp.tile([128, HC, 128], BF16, tag="wk")
                        nc.sync.dma_start(out=wk_t[:], in_=d_wk[:, t, :, :])
                    kpT = p3.tile([128, L], F32R, tag="kpT")
                    for n in range(4):
                        kp_ps = kpps.tile([128, 512], F32, tag="kp_ps")
                        for kc in range(HC):
                            nc.tensor.matmul(kp_ps[:],
                                             wk_t[:, kc, :],
                                             kvT[:, kc, n * 4:(n + 1) * 4, :],
                                             start=(kc == 0), stop=(kc == HC - 1))
                        nc.scalar.activation(out=kpT[:, n * 512:(n + 1) * 512],
                                             in_=kp_ps[:],
                                             func=mybir.ActivationFunctionType.Identity,
                                             bias=bkt_t[:, t:t + 1])
                    for hh in range(2):
                        h = 2 * t + hh
                        hb = hh * 64
                        mexp = p3.tile([128, L], F32, tag="mexp")
                        for n in range(4):
                            sc_ps = scps.tile([128, 512], F32, tag="sc_ps")
                            nc.tensor.matmul(sc_ps[:],
                                             qpT[hb:hb + 64, t, :],
                                             kpT[hb:hb + 64, n * 512:(n + 1) * 512],
                                             start=True, stop=True)
                            nc.scalar.activation(out=mexp[:, n * 512:(n + 1) * 512],
                                                 in_=sc_ps[:],
                                                 func=mybir.ActivationFunctionType.Exp)
                        S = p3s.tile([128, 1], F32, tag="S")
                        nc.vector.scalar_tensor_tensor(out=mexp[:], in0=mexp[:],
                                                       scalar=1.0, in1=maskB[:],
                                                       op0=mybir.AluOpType.mult,
                                                       op1=mybir.AluOpType.mult,
                                                       accum_out=S[:])
                        rS = p3s.tile([128, 1], F32, tag="rS")
                        nc.vector.reciprocal(out=rS[:], in_=S[:])
                        attn_sb = p3.tile([128, L], F32, tag="attn_sb")
                        attn_bf = p3.tile([128, L], BF16, tag="attn_bf")
                        for n in range(4):
                            sl = slice(n * 512, (n + 1) * 512)
                            nc.scalar.activation(out=attn_sb[:, sl], in_=mexp[:, sl],
                                                 func=mybir.ActivationFunctionType.Copy,
                                                 scale=rS[:])
                            nc.gpsimd.tensor_copy(out=attn_bf[:, sl], in_=attn_sb[:, sl])
                        nc.sync.dma_start(out=d_attn[h], in_=attn_sb[:])
                        expT = p3.tile([128, KT, 128], BF16, tag="expT")
                        for q4 in range(4):
                            tp = tpps.tile([128, 4, 128], BF16, tag="ex_tp")
                            for j in range(4):
                                c = q4 * 4 + j
                                nc.tensor.transpose(tp[:, j, :],
                                                    attn_bf[:, c * 128:(c + 1) * 128],
                                                    identb[:])
                            nc.vector.tensor_copy(
                                out=expT[:, q4 * 4:(q4 + 1) * 4, :], in_=tp[:])
                        ctx_ps = ctxps.tile([64, 128], F32, tag="ctx_ps")
                        for c in range(KT):
                            nc.tensor.matmul(ctx_ps[:],
                                             vp[:, c, h * HD:(h + 1) * HD],
                                             expT[:, c, :],
                                             start=(c == 0), stop=(c == KT - 1))
                        nc.vector.tensor_copy(out=ctxT[0:64, h, :], in_=ctx_ps[:])
                        for nn in range(2):
                            nc.tensor.matmul(op_ps[:, nn * 512:(nn + 1) * 512],
                                             ctxT[0:64, h, :],
                                             wo_t[0:64, h, nn * 512:(nn + 1) * 512],
                                             start=(h == 0), stop=(h == NH - 1))

                for cm in reversed(loop_ps_cm):
                    cm.__exit__(None, None, None)

                # =========== P4: residual + out-LN + pooled ===========
                nc.sync.dma_start(out=qmp_t[:], in_=d_qmp[:])
                res = wop.tile([128, H], F32, tag="res")
                nc.vector.tensor_tensor(out=res[:], in0=op_ps[:], in1=resbias[:],
                                        op=mybir.AluOpType.add)
                rstd, nmr = ln_stats(lnp, res, H)
                lnout = wop.tile([128, H], F32R, tag="lnout")
                nc.scalar.activation(out=lnout[:], in_=res[:],
                                     func=mybir.ActivationFunctionType.Identity,
                                     bias=nmr[:], scale=rstd[:])
                pooled_sb = wop.tile([1, H], F32, tag="pooled_sb")
                with tc.tile_pool(name="p4ps", bufs=2, space="PSUM") as p4ps:
                    for nn in range(2):
                        pool_ps = p4ps.tile([1, 512], F32, tag="pool_ps")
                        nc.tensor.matmul(pool_ps[:],
                                         qmp_t[:],
                                         lnout[:, nn * 512:(nn + 1) * 512],
                                         start=True, stop=True)
                        nc.scalar.copy(out=pooled_sb[:, nn * 512:(nn + 1) * 512],
                                       in_=pool_ps[:])
                nc.sync.dma_start(out=d_pooled[:], in_=pooled_sb[:])

    nc.compile()
    return nc


def _prep_inputs(dna_tokens, desc_tokens, dna_attention_mask, desc_attention_mask,
                 q_norm_w, q_norm_b, kv_norm_w, kv_norm_b,
                 in_proj_w, in_proj_b, out_proj_w, out_proj_b):
    f32 = np.float32
    Wq, Wk, Wv = (np.asarray(in_proj_w[i * H:(i + 1) * H], f32) for i in range(3))
    bq, bk, bv = (np.asarray(in_proj_b[i * H:(i + 1) * H], f32) for i in range(3))
    kw = np.asarray(kv_norm_w, f32)
    kb = np.asarray(kv_norm_b, f32)

    # fold kv-norm affine into Wk/Wv (k/v = (z*kw + kb) @ W.T + b)
    WkT = (kw[:, None] * Wk.T.astype(f32))            # [h, o]
    WvT = (kw[:, None] * Wv.T.astype(f32))
    bk_f = bk + Wk.astype(f32) @ kb
    bv_f = bv + Wv.astype(f32) @ kb
    WqT = Wq.T.astype(f32).copy()                     # [h, o]

    def shuf_kc(WT):  # [h, o] -> [128, kc, o]
        return np.ascontiguousarray(WT.reshape(HC, 128, H).transpose(1, 0, 2))

    wq_s = shuf_kc(WqT)
    wv_s = shuf_kc(WvT)
    wk_s = np.ascontiguousarray(
        WkT.reshape(HC, 128, NP, 128).transpose(1, 2, 0, 3))
    WoT = out_proj_w.T.astype(f32)                    # [ho, o]
    wo_s = np.ascontiguousarray(WoT.reshape(NH, 64, H).transpose(1, 0, 2))

    def to_bf16(x):
        import jax.numpy as jnp
        return np.asarray(jnp.asarray(x, dtype=jnp.bfloat16))

    bkt = np.ascontiguousarray(bk_f.reshape(HC, 128).T)  # [128, kc]

    common = {
        "wq": to_bf16(wq_s), "wv": to_bf16(wv_s), "wk": to_bf16(wk_s),
        "wo": to_bf16(wo_s),
        "bq": (bq * SCALE).astype(f32), "bkt": bkt, "bv": bv_f.astype(f32),
        "bo": np.asarray(out_proj_b, f32),
        "wqn": np.asarray(q_norm_w, f32), "bqn": np.asarray(q_norm_b, f32),
        "ident_f32": np.eye(128, dtype=f32),
        "ident_bf16": to_bf16(np.eye(128, dtype=f32)),
    }
    in_maps = []
    for b in range(B):
        qm = np.asarray(desc_attention_mask[b], f32)
        den = max(float(qm.sum()), 1e-8)
        m = dict(common)
        m["desc"] = np.ascontiguousarray(np.asarray(desc_tokens[b], f32))
        m["dna"] = np.ascontiguousarray(np.asarray(dna_tokens[b], f32))
        m["kmask"] = to_bf16(np.asarray(dna_attention_mask[b], f32))
        m["qmask"] = qm[:, None].copy()
        m["qmask_pool"] = (qm / den)[:, None].astype(f32)
        in_maps.append(m)
    return in_maps


def kernel(**inputs):
    if "nc" not in _CACHE:
        _CACHE["nc"] = _build_program()
    nc = _CACHE["nc"]

    onw = np.asarray(inputs["out_norm_w"], np.float32)
    onb = np.asarray(inputs["out_norm_b"], np.float32)
    in_maps = _prep_inputs(
        inputs["dna_tokens"], inputs["desc_tokens"],
        inputs["dna_attention_mask"], inputs["desc_attention_mask"],
        inputs["q_norm_w"], inputs["q_norm_b"],
        inputs["kv_norm_w"], inputs["kv_norm_b"],
        inputs["in_proj_w"], inputs["in_proj_b"],
        inputs["out_proj_w"], inputs["out_proj_b"])

    res = None
    for attempt in range(3):
        try:
            res = run_bass_kernel_spmd(nc, in_maps, list(range(B)))
            break
        except Exception:
            if attempt == 2:
                raise
    assert res is not None
    pooled = np.stack([res.results[b]["pooled"][0] for b in range(B)])
    pooled = pooled * onw[None, :] + onb[None, :]
    attn = np.stack([res.results[b]["attn"] for b in range(B)])
    return pooled.astype(np.float32), attn.astype(np.float32)


# revision 27
# speedup vs baseline: 1.0096x; 1.0096x over previous
"""Trainium2 Bass kernel for nn_CrossAttentionCLSHead.

B=8, L=2048, D=128, H=1024, NH=16, HD=64. Data-parallel: one batch per core.

Per-core pipeline:
  P1: LN(desc) -> qn (q-norm affine + qmask), qT (PE transpose, bf16),
      qp = qT.T @ Wq' (bf16 matmul, f32 psum; 1/8 scale + bias in evac),
      qpT (PE transpose, f32r).
  P2: per k-tile: LN(dna) (kv-norm folded into weights on host) -> z (bf16),
      PE-transpose -> kvT[h,k] (bf16); vp = kvT.T @ Wv' + bv (bf16).
  P3: per head-pair t: kpT[:, t-cols] = Wk'.T @ kvT (+bk, evac f32r);
      per head: scores = qpT.T @ kpT (f32r, K=64), exp (ACT, f32),
      mask-mult + row-sum S (DVE stt w/ accum), attn = mexp * (1/S) -> f32 DMA
      out + bf16 copy (gpsimd); PE-transpose attn_bf -> expT (bf16);
      ctx[hd,q] = vp.T @ expT (bf16, head-major base-0 psum);
      out_proj accumulation for this head (K=64, head-major WoT, bf16).
  P4: res = op_psum + (bo + qn), out-LN, pooled = qmask_pool.T @ ln_out (f32r);
      out-norm affine applied on host.
"""

import sys

if "/opt/trn_rl_repo" not in sys.path:
    sys.path.insert(0, "/opt/trn_rl_repo")

import numpy as np

import concourse.bacc as bacc
import concourse.bass as bass
import concourse.tile as tile
from concourse import mybir
from concourse.bass_utils import run_bass_kernel_spmd

B, L, D, H, NH = 8, 2048, 128, 1024, 16
HD = H // NH          # 64
KT = L // 128         # 16 k-tiles
HC = H // 128         # 8 h-chunks
NP = NH // 2          # 8 head pairs
EPS = 1e-5
SCALE = 1.0 / np.sqrt(HD)

F32 = mybir.dt.float32
F32R = mybir.dt.float32r
BF16 = mybir.dt.bfloat16

_CACHE = {}


def _build_program():
    nc = bacc.Bacc("TRN2", target_bir_lowering=False, debug=False, num_devices=B)

    # ---- DRAM I/O (per core) ----
    d_desc = nc.dram_tensor("desc", [D, H], F32, kind="ExternalInput")
    d_dna = nc.dram_tensor("dna", [L, H], F32, kind="ExternalInput")
    d_wq = nc.dram_tensor("wq", [128, HC, H], BF16, kind="ExternalInput")
    d_wv = nc.dram_tensor("wv", [128, HC, H], BF16, kind="ExternalInput")
    d_wk = nc.dram_tensor("wk", [128, NP, HC, 128], BF16, kind="ExternalInput")
    d_wo = nc.dram_tensor("wo", [64, NH, H], BF16, kind="ExternalInput")
    d_bq = nc.dram_tensor("bq", [H], F32, kind="ExternalInput")  # pre-scaled
    d_bkt = nc.dram_tensor("bkt", [128, HC], F32, kind="ExternalInput")
    d_bv = nc.dram_tensor("bv", [H], F32, kind="ExternalInput")
    d_bo = nc.dram_tensor("bo", [H], F32, kind="ExternalInput")
    d_wqn = nc.dram_tensor("wqn", [H], F32, kind="ExternalInput")
    d_bqn = nc.dram_tensor("bqn", [H], F32, kind="ExternalInput")
    d_km = nc.dram_tensor("kmask", [L], BF16, kind="ExternalInput")
    d_qm = nc.dram_tensor("qmask", [D, 1], F32, kind="ExternalInput")
    d_qmp = nc.dram_tensor("qmask_pool", [D, 1], F32R, kind="ExternalInput")
    d_idf = nc.dram_tensor("ident_f32", [128, 128], F32, kind="ExternalInput")
    d_idb = nc.dram_tensor("ident_bf16", [128, 128], BF16, kind="ExternalInput")
    d_attn = nc.dram_tensor("attn", [NH, D, L], F32, kind="ExternalOutput")
    d_pooled = nc.dram_tensor("pooled", [1, H], F32, kind="ExternalOutput")

    def ln_stats(tc_pool, x, n_free):
        """(rstd, neg_mu_rstd) [128,1] f32 for LN over the free dim."""
        nsub = n_free // 512
        stats = tc_pool.tile([128, nsub, 6], F32, tag="ln_stats")
        for s in range(nsub):
            nc.vector.bn_stats(out=stats[:, s, :], in_=x[:, s * 512:(s + 1) * 512])
        mv = tc_pool.tile([128, 2], F32, tag="ln_mv")
        nc.vector.bn_aggr(out=mv[:], in_=stats[:])
        rstd = tc_pool.tile([128, 1], F32, tag="ln_rstd")
        nc.scalar.activation(out=rstd[:], in_=mv[:, 1:2],
                             func=mybir.ActivationFunctionType.Sqrt, bias=eps_t[:])
        nc.vector.reciprocal(out=rstd[:], in_=rstd[:])
        nmr = tc_pool.tile([128, 1], F32, tag="ln_nmr")
        nc.vector.scalar_tensor_tensor(out=nmr[:], in0=mv[:, 0:1], scalar=-1.0,
                                       in1=rstd[:], op0=mybir.AluOpType.mult,
                                       op1=mybir.AluOpType.mult)
        return rstd, nmr

    with tile.TileContext(nc) as tc:
        from contextlib import ExitStack
        es = ExitStack()
        with es:
            singles = es.enter_context(tc.tile_pool(name="singles", bufs=1))
            persist = es.enter_context(tc.tile_pool(name="persist", bufs=1))
            lnp = es.enter_context(tc.tile_pool(name="lnp", bufs=3))
            wkp = es.enter_context(tc.tile_pool(name="wkp", bufs=2))

            p2w_cm = tc.tile_pool(name="p2w", bufs=1)
            p2w = p2w_cm.__enter__()
            xkp_cm = tc.tile_pool(name="xkp", bufs=4)
            xkp = xkp_cm.__enter__()
            p1_cm = tc.tile_pool(name="p1", bufs=1)
            p1 = p1_cm.__enter__()

            # ---- DMA priority order: q-path first, dna interleaved ----
            xq = p1.tile([128, H], F32)
            nc.sync.dma_start(out=xq[:], in_=d_desc[:])
            wqn_r = p1.tile([1, H], F32)
            nc.sync.dma_start(out=wqn_r[:], in_=d_wqn.ap().unsqueeze(0))
            bqn_r = p1.tile([1, H], F32)
            nc.sync.dma_start(out=bqn_r[:], in_=d_bqn.ap().unsqueeze(0))
            bv_r = singles.tile([1, H], F32)
            nc.sync.dma_start(out=bv_r[:], in_=d_bv.ap().unsqueeze(0))
            bq_r = p1.tile([1, H], F32)
            nc.sync.dma_start(out=bq_r[:], in_=d_bq.ap().unsqueeze(0))
            bo_r = p1.tile([1, H], F32)
            nc.sync.dma_start(out=bo_r[:], in_=d_bo.ap().unsqueeze(0))
            qm_t = singles.tile([128, 1], F32)
            nc.sync.dma_start(out=qm_t[:], in_=d_qm[:])
            bkt_t = singles.tile([128, HC], F32)
            nc.sync.dma_start(out=bkt_t[:], in_=d_bkt[:])
            km_r = p1.tile([1, L], BF16)
            nc.sync.dma_start(out=km_r[:], in_=d_km.ap().unsqueeze(0))
            ident = singles.tile([128, 128], F32)
            nc.sync.dma_start(out=ident[:], in_=d_idf[:])
            identb = singles.tile([128, 128], BF16)
            nc.sync.dma_start(out=identb[:], in_=d_idb[:])
            wq_t = p1.tile([128, HC, H], BF16)
            nc.sync.dma_start(out=wq_t[:, 0:4, :], in_=d_wq[:, 0:4, :])
            xk_pre = {}
            for i in range(2):
                xk = xkp.tile([128, H], F32, tag="xk")
                nc.sync.dma_start(out=xk[:], in_=d_dna[i * 128:(i + 1) * 128, :])
                xk_pre[i] = xk
            wv_t = p2w.tile([128, HC, H], BF16)
            nc.sync.dma_start(out=wv_t[:, 0:4, :], in_=d_wv[:, 0:4, :])
            nc.sync.dma_start(out=wq_t[:, 4:8, :], in_=d_wq[:, 4:8, :])
            xk = xkp.tile([128, H], F32, tag="xk")
            nc.sync.dma_start(out=xk[:], in_=d_dna[2 * 128:3 * 128, :])
            xk_pre[2] = xk
            nc.sync.dma_start(out=wv_t[:, 4:8, :], in_=d_wv[:, 4:8, :])
            wk0 = wkp.tile([128, HC, 128], BF16, tag="wk")
            nc.sync.dma_start(out=wk0[:], in_=d_wk[:, 0, :, :])

            # ---- constants / broadcasts on idle gpsimd ----
            eps_t = singles.tile([128, 1], F32)
            nc.vector.memset(eps_t[:], EPS)
            wqn_t = p1.tile([128, H], F32)
            nc.gpsimd.partition_broadcast(wqn_t[:], wqn_r[:])
            bqn_t = p1.tile([128, H], F32)
            nc.gpsimd.partition_broadcast(bqn_t[:], bqn_r[:])
            bq_t = p1.tile([128, H], F32)
            nc.gpsimd.partition_broadcast(bq_t[:], bq_r[:])
            maskB = singles.tile([128, L], BF16)
            nc.gpsimd.partition_broadcast(maskB[:], km_r[:])
            qmp_t = singles.tile([128, 1], F32R)

            # ---- persistent intermediates ----
            kvT = persist.tile([128, HC, KT, 128], BF16)   # 32KB/p
            vp = persist.tile([128, KT, H], BF16)          # 32KB/p
            qpT = persist.tile([128, HC, 128], F32R)       # 4KB/p
            qn = persist.tile([128, H], F32)               # 4KB/p (residual)
            resbias = persist.tile([128, H], F32)          # 4KB/p
            ctxT = persist.tile([64, NH, 128], BF16)

            # =========== P1: q path ===========
            with tc.tile_pool(name="p1ps", bufs=2, space="PSUM") as p1ps:
                rstd, nmr = ln_stats(lnp, xq, H)
                zq = p1.tile([128, H], F32)
                nc.scalar.activation(out=zq[:], in_=xq[:],
                                     func=mybir.ActivationFunctionType.Identity,
                                     bias=nmr[:], scale=rstd[:])
                # qn = (z*wqn + bqn) * qmask
                bqnm = p1.tile([128, H], F32)
                nc.vector.tensor_scalar(out=bqnm[:], in0=bqn_t[:], scalar1=qm_t[:],
                                        scalar2=None, op0=mybir.AluOpType.mult)
                t1 = p1.tile([128, H], F32)
                nc.vector.tensor_tensor(out=t1[:], in0=zq[:], in1=wqn_t[:],
                                        op=mybir.AluOpType.mult)
                nc.vector.scalar_tensor_tensor(out=qn[:], in0=t1[:], scalar=qm_t[:],
                                               in1=bqnm[:], op0=mybir.AluOpType.mult,
                                               op1=mybir.AluOpType.add)
                bo_t = p1.tile([128, H], F32)
                nc.gpsimd.partition_broadcast(bo_t[:], bo_r[:])
                nc.vector.tensor_tensor(out=resbias[:], in0=bo_t[:], in1=qn[:],
                                        op=mybir.AluOpType.add)
                # qT (bf16) via PE transpose of qn (f32 input; 8 ops)
                qT = p1.tile([128, HC, 128], BF16)
                for t in range(HC):
                    tp = p1ps.tile([128, 128], F32, tag="qt_ps")
                    nc.tensor.transpose(tp[:], qn[:, t * 128:(t + 1) * 128], ident[:])
                    nc.scalar.copy(out=qT[:, t, :], in_=tp[:])
                # qp = qT.T @ wq (bf16, accumulate over kc) -> [128, H] f32 psum
                qp_ps = p1ps.tile([128, H], F32, tag="qp_ps")
                for kc in range(HC):
                    for nn in range(2):
                        nc.tensor.matmul(qp_ps[:, nn * 512:(nn + 1) * 512],
                                         qT[:, kc, :],
                                         wq_t[:, kc, nn * 512:(nn + 1) * 512],
                                         start=(kc == 0), stop=(kc == HC - 1))
                qp_sb = p1.tile([128, H], F32)
                nc.vector.scalar_tensor_tensor(out=qp_sb[:], in0=qp_ps[:],
                                               scalar=float(SCALE), in1=bq_t[:],
                                               op0=mybir.AluOpType.mult,
                                               op1=mybir.AluOpType.add)
                for t in range(HC):
                    tp = p1ps.tile([128, 128], F32, tag="qt_ps")
                    nc.tensor.transpose(tp[:], qp_sb[:, t * 128:(t + 1) * 128], ident[:])
                    nc.scalar.copy(out=qpT[:, t, :], in_=tp[:])
            p1_cm.__exit__(None, None, None)

            # =========== P2: kv path ===========
            with tc.tile_pool(name="p2", bufs=2) as p2, \
                 tc.tile_pool(name="p2ps", bufs=2, space="PSUM") as p2ps, \
                 tc.tile_pool(name="p2tp", bufs=2, space="PSUM") as p2tp:
                bv_t = p2w.tile([128, H], F32)
                nc.gpsimd.partition_broadcast(bv_t[:], bv_r[:])
                for i in range(KT):
                    if i in xk_pre:
                        xk = xk_pre.pop(i)
                    else:
                        xk = xkp.tile([128, H], F32, tag="xk")
                        nc.sync.dma_start(out=xk[:], in_=d_dna[i * 128:(i + 1) * 128, :])
                    rstd, nmr = ln_stats(lnp, xk, H)
                    zk = p2.tile([128, H], BF16, tag="zk")
                    nc.scalar.activation(out=zk[:], in_=xk[:],
                                         func=mybir.ActivationFunctionType.Identity,
                                         bias=nmr[:], scale=rstd[:])
                    for half in range(2):
                        tp = p2tp.tile([128, 4, 128], BF16, tag="kv_tp")
                        for j in range(4):
                            t = half * 4 + j
                            nc.tensor.transpose(tp[:, j, :],
                                                zk[:, t * 128:(t + 1) * 128], identb[:])
                        nc.scalar.copy(
                            out=kvT[:, half * 4:(half + 1) * 4, i, :], in_=tp[:])
                    vp_ps = p2ps.tile([128, H], F32, tag="vp_ps")
                    for kc in range(HC):
                        for nn in range(2):
                            nc.tensor.matmul(vp_ps[:, nn * 512:(nn + 1) * 512],
                                             kvT[:, kc, i, :],
                                             wv_t[:, kc, nn * 512:(nn + 1) * 512],
                                             start=(kc == 0), stop=(kc == HC - 1))
                    nc.vector.tensor_tensor(out=vp[:, i, :], in0=vp_ps[:], in1=bv_t[:],
                                            op=mybir.AluOpType.add)
            xkp_cm.__exit__(None, None, None)
            p2w_cm.__exit__(None, None, None)

            # =========== P3: head loop (+ out_proj accumulation) ===========
            with tc.tile_pool(name="p3", bufs=2) as p3, \
                 tc.tile_pool(name="p3s", bufs=4) as p3s, \
                 tc.tile_pool(name="wop", bufs=1) as wop:
                wo_t = wop.tile([64, NH, H], BF16)
                nc.sync.dma_start(out=wo_t[:], in_=d_wo[:])
                loop_ps_cm = [tc.tile_pool(name="kpps", bufs=2, space="PSUM"),
                              tc.tile_pool(name="scps", bufs=2, space="PSUM"),
                              tc.tile_pool(name="tpps", bufs=2, space="PSUM"),
                              tc.tile_pool(name="ctxps", bufs=2, space="PSUM")]
                kpps, scps, tpps, ctxps = [cm.__enter__() for cm in loop_ps_cm]
                for t in range(NP):
                    if t == 0:
                        wk_t = wk0
                    else:
                        wk_t = wkp.tile([128, HC, 128], BF16, tag="wk")
                        nc.sync.dma_start(out=wk_t[:], in_=d_wk[:, t, :, :])
                    kpT = p3.tile([128, L], F32R, tag="kpT")
                    for n in range(4):
                        kp_ps = kpps.tile([128, 512], F32, tag="kp_ps")
                        for kc in range(HC):
                            nc.tensor.matmul(kp_ps[:],
                                             wk_t[:, kc, :],
                                             kvT[:, kc, n * 4:(n + 1) * 4, :],
                                             start=(kc == 0), stop=(kc == HC - 1))
                        nc.scalar.activation(out=kpT[:, n * 512:(n + 1) * 512],
                                             in_=kp_ps[:],
                                             func=mybir.ActivationFunctionType.Identity,
                                             bias=bkt_t[:, t:t + 1])
                    for hh in range(2):
                        h = 2 * t + hh
                        hb = hh * 64
                        mexp = p3.tile([128, L], F32, tag="mexp")
                        for n in range(4):
                            sc_ps = scps.tile([128, 512], F32, tag="sc_ps")
                            nc.tensor.matmul(sc_ps[:],
                                             qpT[hb:hb + 64, t, :],
                                             kpT[hb:hb + 64, n * 512:(n + 1) * 512],
                                             start=True, stop=True)
                            nc.scalar.activation(out=mexp[:, n * 512:(n + 1) * 512],
                                                 in_=sc_ps[:],
                                                 func=mybir.ActivationFunctionType.Exp)
                        S = p3s.tile([128, 1], F32, tag="S")
                        nc.vector.scalar_tensor_tensor(out=mexp[:], in0=mexp[:],
                                                       scalar=1.0, in1=maskB[:],
                                                       op0=mybir.AluOpType.mult,
                                                       op1=mybir.AluOpType.mult,
                                                       accum_out=S[:])
                        rS = p3s.tile([128, 1], F32, tag="rS")
                        nc.vector.reciprocal(out=rS[:], in_=S[:])
                        attn_sb = p3.tile([128, L], F32, tag="attn_sb")
                        attn_bf = p3.tile([128, L], BF16, tag="attn_bf")
                        for n in range(4):
                            sl = slice(n * 512, (n + 1) * 512)
                            nc.scalar.activation(out=attn_sb[:, sl], in_=mexp[:, sl],
                                                 func=mybir.ActivationFunctionType.Copy,
                                                 scale=rS[:])
                            nc.gpsimd.tensor_copy(out=attn_bf[:, sl], in_=attn_sb[:, sl])
                        nc.sync.dma_start(out=d_attn[h], in_=attn_sb[:])
                        expT = p3.tile([128, KT, 128], BF16, tag="expT")
                        for q4 in range(4):
                            tp = tpps.tile([128, 4, 128], BF16, tag="ex_tp")
                            for j in range(4):
                                c = q4 * 4 + j
                                nc.tensor.transpose(tp[:, j, :],
                                                    attn_bf[:, c * 128:(c + 1) * 128],
                                                    identb[:])
                            nc.vector.tensor_copy(
                                out=expT[:, q4 * 4:(q4 + 1) * 4, :], in_=tp[:])
                        ctx_ps = ctxps.tile([64, 128], F32, tag="ctx_ps")
                        for c in range(KT):
                            nc.tensor.matmul(ctx_ps[:],
                                             vp[:, c, h * HD:(h + 1) * HD],
                                             expT[:, c, :],
                                             start=(c == 0), stop=(c == KT - 1))
                        nc.vector.tensor_copy(out=ctxT[0:64, h, :], in_=ctx_ps[:])

                for cm in reversed(loop_ps_cm):
                    cm.__exit__(None, None, None)

                # =========== P4: out_proj + residual + out-LN + pooled ===========
                nc.sync.dma_start(out=qmp_t[:], in_=d_qmp[:])
                opps_cm = tc.tile_pool(name="opps", bufs=1, space="PSUM")
                opps = opps_cm.__enter__()
                op_ps = opps.tile([128, H], F32)
                for h in range(NH):
                    for nn in range(2):
                        nc.tensor.matmul(op_ps[:, nn * 512:(nn + 1) * 512],
                                         ctxT[0:64, h, :],
                                         wo_t[0:64, h, nn * 512:(nn + 1) * 512],
                                         start=(h == 0), stop=(h == NH - 1))
                res = wop.tile([128, H], F32, tag="res")
                nc.vector.tensor_tensor(out=res[:], in0=op_ps[:], in1=resbias[:],
                                        op=mybir.AluOpType.add)
                opps_cm.__exit__(None, None, None)
                rstd, nmr = ln_stats(lnp, res, H)
                lnout = wop.tile([128, H], F32R, tag="lnout")
                nc.scalar.activation(out=lnout[:], in_=res[:],
                                     func=mybir.ActivationFunctionType.Identity,
                                     bias=nmr[:], scale=rstd[:])
                pooled_sb = wop.tile([1, H], F32, tag="pooled_sb")
                with tc.tile_pool(name="p4ps", bufs=2, space="PSUM") as p4ps:
                    for nn in range(2):
                        pool_ps = p4ps.tile([1, 512], F32, tag="pool_ps")
                        nc.tensor.matmul(pool_ps[:],
                                         qmp_t[:],
                                         lnout[:, nn * 512:(nn + 1) * 512],
                                         start=True, stop=True)
                        nc.scalar.copy(out=pooled_sb[:, nn * 512:(nn + 1) * 512],
                                       in_=pool_ps[:])
                nc.sync.dma_start(out=d_pooled[:], in_=pooled_sb[:])

    nc.compile()
    return nc


def _prep_inputs(dna_tokens, desc_tokens, dna_attention_mask, desc_attention_mask,
                 q_norm_w, q_norm_b, kv_norm_w, kv_norm_b,
                 in_proj_w, in_proj_b, out_proj_w, out_proj_b):
    f32 = np.float32
    Wq, Wk, Wv = (np.asarray(in_proj_w[i * H:(i + 1) * H], f32) for i in range(3))
    bq, bk, bv = (np.asarray(in_proj_b[i * H:(i + 1) * H], f32) for i in range(3))
    kw = np.asarray(kv_norm_w, f32)
    kb = np.asarray(kv_norm_b, f32)

    # fold kv-norm affine into Wk/Wv (k/v = (z*kw + kb) @ W.T + b)
    WkT = (kw[:, None] * Wk.T.astype(f32))            # [h, o]
    WvT = (kw[:, None] * Wv.T.astype(f32))
    bk_f = bk + Wk.astype(f32) @ kb
    bv_f = bv + Wv.astype(f32) @ kb
    WqT = Wq.T.astype(f32).copy()                     # [h, o]

    def shuf_kc(WT):  # [h, o] -> [128, kc, o]
        return np.ascontiguousarray(WT.reshape(HC, 128, H).transpose(1, 0, 2))

    wq_s = shuf_kc(WqT)
    wv_s = shuf_kc(WvT)
    wk_s = np.ascontiguousarray(
        WkT.reshape(HC, 128, NP, 128).transpose(1, 2, 0, 3))
    WoT = out_proj_w.T.astype(f32)                    # [ho, o]
    wo_s = np.ascontiguousarray(WoT.reshape(NH, 64, H).transpose(1, 0, 2))

    def to_bf16(x):
        import jax.numpy as jnp
        return np.asarray(jnp.asarray(x, dtype=jnp.bfloat16))

    bkt = np.ascontiguousarray(bk_f.reshape(HC, 128).T)  # [128, kc]

    common = {
        "wq": to_bf16(wq_s), "wv": to_bf16(wv_s), "wk": to_bf16(wk_s),
        "wo": to_bf16(wo_s),
        "bq": (bq * SCALE).astype(f32), "bkt": bkt, "bv": bv_f.astype(f32),
        "bo": np.asarray(out_proj_b, f32),
        "wqn": np.asarray(q_norm_w, f32), "bqn": np.asarray(q_norm_b, f32),
        "ident_f32": np.eye(128, dtype=f32),
        "ident_bf16": to_bf16(np.eye(128, dtype=f32)),
    }
    in_maps = []
    for b in range(B):
        qm = np.asarray(desc_attention_mask[b], f32)
        den = max(float(qm.sum()), 1e-8)
        m = dict(common)
        m["desc"] = np.ascontiguousarray(np.asarray(desc_tokens[b], f32))
        m["dna"] = np.ascontiguousarray(np.asarray(dna_tokens[b], f32))
        m["kmask"] = to_bf16(np.asarray(dna_attention_mask[b], f32))
        m["qmask"] = qm[:, None].copy()
        m["qmask_pool"] = (qm / den)[:, None].astype(f32)
        in_maps.append(m)
    return in_maps


def kernel(**inputs):
    if "nc" not in _CACHE:
        _CACHE["nc"] = _build_program()
    nc = _CACHE["nc"]

    onw = np.asarray(inputs["out_norm_w"], np.float32)
    onb = np.asarray(inputs["out_norm_b"], np.float32)
    in_maps = _prep_inputs(
        inputs["dna_tokens"], inputs["desc_tokens"],
        inputs["dna_attention_mask"], inputs["desc_attention_mask"],
        inputs["q_norm_w"], inputs["q_norm_b"],
        inputs["kv_norm_w"], inputs["kv_norm_b"],
        inputs["in_proj_w"], inputs["in_proj_b"],
        inputs["out_proj_w"], inputs["out_proj_b"])

    res = None
    for attempt in range(3):
        try:
            res = run_bass_kernel_spmd(nc, in_maps, list(range(B)))
            break
        except Exception:
            if attempt == 2:
                raise
    assert res is not None
    pooled = np.stack([res.results[b]["pooled"][0] for b in range(B)])
    pooled = pooled * onw[None, :] + onb[None, :]
    attn = np.stack([res.results[b]["attn"] for b in range(B)])
    return pooled.astype(np.float32), attn.astype(np.float32)


# revision 33
# speedup vs baseline: 1.0119x; 1.0023x over previous
"""Trainium2 Bass kernel for nn_CrossAttentionCLSHead.

B=8, L=2048, D=128, H=1024, NH=16, HD=64. Data-parallel: one batch per core.

Per-core pipeline:
  P1: LN(desc) -> qn (q-norm affine + qmask), qT (PE transpose, bf16),
      qp = qT.T @ Wq' (bf16 matmul, f32 psum; 1/8 scale + bias in evac),
      qpT (PE transpose, f32r).
  P2: per k-tile: LN(dna) (kv-norm folded into weights on host) -> z (bf16),
      PE-transpose -> kvT[h,k] (bf16); vp = kvT.T @ Wv' + bv (bf16).
  P3: per head-pair t: kpT[:, t-cols] = Wk'.T @ kvT (+bk, evac f32r);
      per head: scores = qpT.T @ kpT (f32r, K=64), exp (ACT, f32),
      mask-mult + row-sum S (DVE stt w/ accum), attn = mexp * (1/S) -> f32 DMA
      out + bf16 copy (gpsimd); PE-transpose attn_bf -> expT (bf16);
      ctx[hd,q] = vp.T @ expT (bf16, head-major base-0 psum);
      out_proj accumulation for this head (K=64, head-major WoT, bf16).
  P4: res = op_psum + (bo + qn), out-LN, pooled = qmask_pool.T @ ln_out (f32r);
      out-norm affine applied on host.
"""

import sys

if "/opt/trn_rl_repo" not in sys.path:
    sys.path.insert(0, "/opt/trn_rl_repo")

import numpy as np

import concourse.bacc as bacc
import concourse.bass as bass
import concourse.tile as tile
from concourse import mybir
from concourse.bass_utils import run_bass_kernel_spmd

B, L, D, H, NH = 8, 2048, 128, 1024, 16
HD = H // NH          # 64
KT = L // 128         # 16 k-tiles
HC = H // 128         # 8 h-chunks
NP = NH // 2          # 8 head pairs
EPS = 1e-5
SCALE = 1.0 / np.sqrt(HD)

F32 = mybir.dt.float32
F32R = mybir.dt.float32r
BF16 = mybir.dt.bfloat16

_CACHE = {}


def _build_program():
    nc = bacc.Bacc("TRN2", target_bir_lowering=False, debug=False, num_devices=B)

    # ---- DRAM I/O (per core) ----
    d_desc = nc.dram_tensor("desc", [D, H], F32, kind="ExternalInput")
    d_dna = nc.dram_tensor("dna", [L, H], F32, kind="ExternalInput")
    d_wq = nc.dram_tensor("wq", [128, HC, H], BF16, kind="ExternalInput")
    d_wv = nc.dram_tensor("wv", [128, HC, H], BF16, kind="ExternalInput")
    d_wk = nc.dram_tensor("wk", [128, NP, HC, 128], BF16, kind="ExternalInput")
    d_wo = nc.dram_tensor("wo", [64, NH, H], BF16, kind="ExternalInput")
    d_bq = nc.dram_tensor("bq", [H], F32, kind="ExternalInput")  # pre-scaled
    d_bkt = nc.dram_tensor("bkt", [128, HC], F32, kind="ExternalInput")
    d_bv = nc.dram_tensor("bv", [H], F32, kind="ExternalInput")
    d_bo = nc.dram_tensor("bo", [H], F32, kind="ExternalInput")
    d_wqn = nc.dram_tensor("wqn", [H], F32, kind="ExternalInput")
    d_bqn = nc.dram_tensor("bqn", [H], F32, kind="ExternalInput")
    d_km = nc.dram_tensor("kmask", [L], BF16, kind="ExternalInput")
    d_qm = nc.dram_tensor("qmask", [D, 1], F32, kind="ExternalInput")
    d_qmp = nc.dram_tensor("qmask_pool", [D, 1], F32R, kind="ExternalInput")
    d_idf = nc.dram_tensor("ident_f32", [128, 128], F32, kind="ExternalInput")
    d_idb = nc.dram_tensor("ident_bf16", [128, 128], BF16, kind="ExternalInput")
    d_attn = nc.dram_tensor("attn", [NH, D, L], F32, kind="ExternalOutput")
    d_pooled = nc.dram_tensor("pooled", [1, H], F32, kind="ExternalOutput")

    def ln_stats(tc_pool, x, n_free):
        """(rstd, neg_mu_rstd) [128,1] f32 for LN over the free dim."""
        nsub = n_free // 512
        stats = tc_pool.tile([128, nsub, 6], F32, tag="ln_stats")
        for s in range(nsub):
            nc.vector.bn_stats(out=stats[:, s, :], in_=x[:, s * 512:(s + 1) * 512])
        mv = tc_pool.tile([128, 2], F32, tag="ln_mv")
        nc.vector.bn_aggr(out=mv[:], in_=stats[:])
        rstd = tc_pool.tile([128, 1], F32, tag="ln_rstd")
        nc.scalar.activation(out=rstd[:], in_=mv[:, 1:2],
                             func=mybir.ActivationFunctionType.Sqrt, bias=eps_t[:])
        nc.vector.reciprocal(out=rstd[:], in_=rstd[:])
        nmr = tc_pool.tile([128, 1], F32, tag="ln_nmr")
        nc.vector.scalar_tensor_tensor(out=nmr[:], in0=mv[:, 0:1], scalar=-1.0,
                                       in1=rstd[:], op0=mybir.AluOpType.mult,
                                       op1=mybir.AluOpType.mult)
        return rstd, nmr

    with tile.TileContext(nc) as tc:
        from contextlib import ExitStack
        es = ExitStack()
        with es:
            singles = es.enter_context(tc.tile_pool(name="singles", bufs=1))
            persist = es.enter_context(tc.tile_pool(name="persist", bufs=1))
            lnp = es.enter_context(tc.tile_pool(name="lnp", bufs=3))
            wkp = es.enter_context(tc.tile_pool(name="wkp", bufs=3))

            p2w_cm = tc.tile_pool(name="p2w", bufs=1)
            p2w = p2w_cm.__enter__()
            xkp_cm = tc.tile_pool(name="xkp", bufs=4)
            xkp = xkp_cm.__enter__()
            p1_cm = tc.tile_pool(name="p1", bufs=1)
            p1 = p1_cm.__enter__()

            # ---- DMA priority order: q-path first, dna interleaved ----
            xq = p1.tile([128, H], F32)
            nc.sync.dma_start(out=xq[:], in_=d_desc[:])
            wqn_r = p1.tile([1, H], F32)
            nc.sync.dma_start(out=wqn_r[:], in_=d_wqn.ap().unsqueeze(0))
            bqn_r = p1.tile([1, H], F32)
            nc.sync.dma_start(out=bqn_r[:], in_=d_bqn.ap().unsqueeze(0))
            bv_r = singles.tile([1, H], F32)
            nc.sync.dma_start(out=bv_r[:], in_=d_bv.ap().unsqueeze(0))
            bq_r = p1.tile([1, H], F32)
            nc.sync.dma_start(out=bq_r[:], in_=d_bq.ap().unsqueeze(0))
            bo_r = p1.tile([1, H], F32)
            nc.sync.dma_start(out=bo_r[:], in_=d_bo.ap().unsqueeze(0))
            qm_t = singles.tile([128, 1], F32)
            nc.sync.dma_start(out=qm_t[:], in_=d_qm[:])
            bkt_t = singles.tile([128, HC], F32)
            nc.sync.dma_start(out=bkt_t[:], in_=d_bkt[:])
            km_r = p1.tile([1, L], BF16)
            nc.sync.dma_start(out=km_r[:], in_=d_km.ap().unsqueeze(0))
            ident = singles.tile([128, 128], F32)
            nc.sync.dma_start(out=ident[:], in_=d_idf[:])
            identb = singles.tile([128, 128], BF16)
            nc.sync.dma_start(out=identb[:], in_=d_idb[:])
            wq_t = p1.tile([128, HC, H], BF16)
            nc.sync.dma_start(out=wq_t[:, 0:4, :], in_=d_wq[:, 0:4, :])
            xk_pre = {}
            for i in range(2):
                xk = xkp.tile([128, H], F32, tag="xk")
                nc.sync.dma_start(out=xk[:], in_=d_dna[i * 128:(i + 1) * 128, :])
                xk_pre[i] = xk
            wv_t = p2w.tile([128, HC, H], BF16)
            nc.sync.dma_start(out=wv_t[:, 0:4, :], in_=d_wv[:, 0:4, :])
            nc.sync.dma_start(out=wq_t[:, 4:8, :], in_=d_wq[:, 4:8, :])
            xk = xkp.tile([128, H], F32, tag="xk")
            nc.sync.dma_start(out=xk[:], in_=d_dna[2 * 128:3 * 128, :])
            xk_pre[2] = xk
            nc.sync.dma_start(out=wv_t[:, 4:8, :], in_=d_wv[:, 4:8, :])
            wk0 = wkp.tile([128, HC, 128], BF16, tag="wk")
            nc.sync.dma_start(out=wk0[:], in_=d_wk[:, 0, :, :])

            # ---- constants / broadcasts on idle gpsimd ----
            eps_t = singles.tile([128, 1], F32)
            nc.vector.memset(eps_t[:], EPS)
            wqn_t = p1.tile([128, H], F32)
            nc.gpsimd.partition_broadcast(wqn_t[:], wqn_r[:])
            bqn_t = p1.tile([128, H], F32)
            nc.gpsimd.partition_broadcast(bqn_t[:], bqn_r[:])
            bq_t = p1.tile([128, H], F32)
            nc.gpsimd.partition_broadcast(bq_t[:], bq_r[:])
            maskB = singles.tile([128, L], BF16)
            nc.gpsimd.partition_broadcast(maskB[:], km_r[:])
            qmp_t = singles.tile([128, 1], F32R)

            # ---- persistent intermediates ----
            kvT = persist.tile([128, HC, KT, 128], BF16)   # 32KB/p
            vp = persist.tile([128, KT, H], BF16)          # 32KB/p
            qpT = persist.tile([128, HC, 128], F32R)       # 4KB/p
            qn = persist.tile([128, H], F32)               # 4KB/p (residual)
            resbias = persist.tile([128, H], F32)          # 4KB/p
            ctxT = persist.tile([64, NH, 128], BF16)

            # =========== P1: q path ===========
            with tc.tile_pool(name="p1ps", bufs=2, space="PSUM") as p1ps:
                rstd, nmr = ln_stats(lnp, xq, H)
                zq = p1.tile([128, H], F32)
                nc.scalar.activation(out=zq[:], in_=xq[:],
                                     func=mybir.ActivationFunctionType.Identity,
                                     bias=nmr[:], scale=rstd[:])
                # qn = (z*wqn + bqn) * qmask
                bqnm = p1.tile([128, H], F32)
                nc.vector.tensor_scalar(out=bqnm[:], in0=bqn_t[:], scalar1=qm_t[:],
                                        scalar2=None, op0=mybir.AluOpType.mult)
                t1 = p1.tile([128, H], F32)
                nc.vector.tensor_tensor(out=t1[:], in0=zq[:], in1=wqn_t[:],
                                        op=mybir.AluOpType.mult)
                nc.vector.scalar_tensor_tensor(out=qn[:], in0=t1[:], scalar=qm_t[:],
                                               in1=bqnm[:], op0=mybir.AluOpType.mult,
                                               op1=mybir.AluOpType.add)
                bo_t = p1.tile([128, H], F32)
                nc.gpsimd.partition_broadcast(bo_t[:], bo_r[:])
                nc.vector.tensor_tensor(out=resbias[:], in0=bo_t[:], in1=qn[:],
                                        op=mybir.AluOpType.add)
                # qT (bf16) via PE transpose of qn (f32 input; 8 ops)
                qT = p1.tile([128, HC, 128], BF16)
                for t in range(HC):
                    tp = p1ps.tile([128, 128], F32, tag="qt_ps")
                    nc.tensor.transpose(tp[:], qn[:, t * 128:(t + 1) * 128], ident[:])
                    nc.scalar.copy(out=qT[:, t, :], in_=tp[:])
                # qp = qT.T @ wq (bf16, accumulate over kc) -> [128, H] f32 psum
                qp_ps = p1ps.tile([128, H], F32, tag="qp_ps")
                for kc in range(HC):
                    for nn in range(2):
                        nc.tensor.matmul(qp_ps[:, nn * 512:(nn + 1) * 512],
                                         qT[:, kc, :],
                                         wq_t[:, kc, nn * 512:(nn + 1) * 512],
                                         start=(kc == 0), stop=(kc == HC - 1))
                qp_sb = p1.tile([128, H], F32)
                nc.vector.scalar_tensor_tensor(out=qp_sb[:], in0=qp_ps[:],
                                               scalar=float(SCALE), in1=bq_t[:],
                                               op0=mybir.AluOpType.mult,
                                               op1=mybir.AluOpType.add)
                for t in range(HC):
                    tp = p1ps.tile([128, 128], F32, tag="qt_ps")
                    nc.tensor.transpose(tp[:], qp_sb[:, t * 128:(t + 1) * 128], ident[:])
                    nc.scalar.copy(out=qpT[:, t, :], in_=tp[:])
            p1_cm.__exit__(None, None, None)

            # =========== P2: kv path ===========
            with tc.tile_pool(name="p2", bufs=2) as p2, \
                 tc.tile_pool(name="p2ps", bufs=2, space="PSUM") as p2ps, \
                 tc.tile_pool(name="p2tp", bufs=2, space="PSUM") as p2tp:
                bv_t = p2w.tile([128, H], F32)
                nc.gpsimd.partition_broadcast(bv_t[:], bv_r[:])
                for i in range(KT):
                    if i in xk_pre:
                        xk = xk_pre.pop(i)
                    else:
                        xk = xkp.tile([128, H], F32, tag="xk")
                        nc.sync.dma_start(out=xk[:], in_=d_dna[i * 128:(i + 1) * 128, :])
                    rstd, nmr = ln_stats(lnp, xk, H)
                    zk = p2.tile([128, H], BF16, tag="zk")
                    nc.scalar.activation(out=zk[:], in_=xk[:],
                                         func=mybir.ActivationFunctionType.Identity,
                                         bias=nmr[:], scale=rstd[:])
                    for half in range(2):
                        tp = p2tp.tile([128, 4, 128], BF16, tag="kv_tp")
                        for j in range(4):
                            t = half * 4 + j
                            nc.tensor.transpose(tp[:, j, :],
                                                zk[:, t * 128:(t + 1) * 128], identb[:])
                        nc.scalar.copy(
                            out=kvT[:, half * 4:(half + 1) * 4, i, :], in_=tp[:])
                    vp_ps = p2ps.tile([128, H], F32, tag="vp_ps")
                    for kc in range(HC):
                        for nn in range(2):
                            nc.tensor.matmul(vp_ps[:, nn * 512:(nn + 1) * 512],
                                             kvT[:, kc, i, :],
                                             wv_t[:, kc, nn * 512:(nn + 1) * 512],
                                             start=(kc == 0), stop=(kc == HC - 1))
                    nc.vector.tensor_tensor(out=vp[:, i, :], in0=vp_ps[:], in1=bv_t[:],
                                            op=mybir.AluOpType.add)
            xkp_cm.__exit__(None, None, None)
            p2w_cm.__exit__(None, None, None)

            # =========== P3: head loop (+ out_proj accumulation) ===========
            with tc.tile_pool(name="p3", bufs=2) as p3, \
                 tc.tile_pool(name="p3s", bufs=4) as p3s, \
                 tc.tile_pool(name="wop", bufs=1) as wop:
                wo_t = wop.tile([64, NH, H], BF16)
                nc.sync.dma_start(out=wo_t[:], in_=d_wo[:])
                loop_ps_cm = [tc.tile_pool(name="kpps", bufs=2, space="PSUM"),
                              tc.tile_pool(name="scps", bufs=2, space="PSUM"),
                              tc.tile_pool(name="tpps", bufs=2, space="PSUM"),
                              tc.tile_pool(name="ctxps", bufs=2, space="PSUM")]
                kpps, scps, tpps, ctxps = [cm.__enter__() for cm in loop_ps_cm]
                for t in range(NP):
                    if t == 0:
                        wk_t = wk0
                    else:
                        wk_t = wkp.tile([128, HC, 128], BF16, tag="wk")
                        nc.sync.dma_start(out=wk_t[:], in_=d_wk[:, t, :, :])
                    kpT = p3.tile([128, L], F32R, tag="kpT")
                    for n in range(4):
                        kp_ps = kpps.tile([128, 512], F32, tag="kp_ps")
                        for kc in range(HC):
                            nc.tensor.matmul(kp_ps[:],
                                             wk_t[:, kc, :],
                                             kvT[:, kc, n * 4:(n + 1) * 4, :],
                                             start=(kc == 0), stop=(kc == HC - 1))
                        nc.scalar.activation(out=kpT[:, n * 512:(n + 1) * 512],
                                             in_=kp_ps[:],
                                             func=mybir.ActivationFunctionType.Identity,
                                             bias=bkt_t[:, t:t + 1])
                    for hh in range(2):
                        h = 2 * t + hh
                        hb = hh * 64
                        mexp = p3.tile([128, L], F32, tag="mexp")
                        for n in range(4):
                            sc_ps = scps.tile([128, 512], F32, tag="sc_ps")
                            nc.tensor.matmul(sc_ps[:],
                                             qpT[hb:hb + 64, t, :],
                                             kpT[hb:hb + 64, n * 512:(n + 1) * 512],
                                             start=True, stop=True)
                            nc.scalar.activation(out=mexp[:, n * 512:(n + 1) * 512],
                                                 in_=sc_ps[:],
                                                 func=mybir.ActivationFunctionType.Exp)
                        S = p3s.tile([128, 1], F32, tag="S")
                        nc.vector.scalar_tensor_tensor(out=mexp[:], in0=mexp[:],
                                                       scalar=1.0, in1=maskB[:],
                                                       op0=mybir.AluOpType.mult,
                                                       op1=mybir.AluOpType.mult,
                                                       accum_out=S[:])
                        rS = p3s.tile([128, 1], F32, tag="rS")
                        nc.vector.reciprocal(out=rS[:], in_=S[:])
                        attn_sb = p3.tile([128, L], F32, tag="attn_sb")
                        attn_bf = p3.tile([128, L], BF16, tag="attn_bf")
                        for n in range(4):
                            sl = slice(n * 512, (n + 1) * 512)
                            nc.scalar.activation(out=attn_sb[:, sl], in_=mexp[:, sl],
                                                 func=mybir.ActivationFunctionType.Copy,
                                                 scale=rS[:])
                            nc.gpsimd.tensor_copy(out=attn_bf[:, sl], in_=attn_sb[:, sl])
                        nc.sync.dma_start(out=d_attn[h], in_=attn_sb[:])
                        expT = p3.tile([128, KT, 128], BF16, tag="expT")
                        for q4 in range(4):
                            tp = tpps.tile([128, 4, 128], BF16, tag="ex_tp")
                            for j in range(4):
                                c = q4 * 4 + j
                                nc.tensor.transpose(tp[:, j, :],
                                                    attn_bf[:, c * 128:(c + 1) * 128],
                                                    identb[:])
                            nc.vector.tensor_copy(
                                out=expT[:, q4 * 4:(q4 + 1) * 4, :], in_=tp[:])
                        ctx_ps = ctxps.tile([64, 128], F32, tag="ctx_ps")
                        for c in range(KT):
                            nc.tensor.matmul(ctx_ps[:],
                                             vp[:, c, h * HD:(h + 1) * HD],
                                             expT[:, c, :],
                                             start=(c == 0), stop=(c == KT - 1))
                        nc.vector.tensor_copy(out=ctxT[0:64, h, :], in_=ctx_ps[:])

                for cm in reversed(loop_ps_cm):
                    cm.__exit__(None, None, None)

                # =========== P4: out_proj + residual + out-LN + pooled ===========
                nc.sync.dma_start(out=qmp_t[:], in_=d_qmp[:])
                opps_cm = tc.tile_pool(name="opps", bufs=1, space="PSUM")
                opps = opps_cm.__enter__()
                op_ps = opps.tile([128, H], F32)
                for h in range(NH):
                    for nn in range(2):
                        nc.tensor.matmul(op_ps[:, nn * 512:(nn + 1) * 512],
                                         ctxT[0:64, h, :],
                                         wo_t[0:64, h, nn * 512:(nn + 1) * 512],
                                         start=(h == 0), stop=(h == NH - 1))
                res = wop.tile([128, H], F32, tag="res")
                nc.vector.tensor_tensor(out=res[:], in0=op_ps[:], in1=resbias[:],
                                        op=mybir.AluOpType.add)
                opps_cm.__exit__(None, None, None)
                rstd, nmr = ln_stats(lnp, res, H)
                lnout = wop.tile([128, H], F32R, tag="lnout")
                nc.scalar.activation(out=lnout[:], in_=res[:],
                                     func=mybir.ActivationFunctionType.Identity,
                                     bias=nmr[:], scale=rstd[:])
                pooled_sb = wop.tile([1, H], F32, tag="pooled_sb")
                with tc.tile_pool(name="p4ps", bufs=2, space="PSUM") as p4ps:
                    for nn in range(2):
                        pool_ps = p4ps.tile([1, 512], F32, tag="pool_ps")
                        nc.tensor.matmul(pool_ps[:],
                                         qmp_t[:],
                                         lnout[:, nn * 512:(nn + 1) * 512],
                                         start=True, stop=True)
                        nc.scalar.copy(out=pooled_sb[:, nn * 512:(nn + 1) * 512],
                                       in_=pool_ps[:])
                nc.sync.dma_start(out=d_pooled[:], in_=pooled_sb[:])

    nc.compile()
    return nc


def _prep_inputs(dna_tokens, desc_tokens, dna_attention_mask, desc_attention_mask,
                 q_norm_w, q_norm_b, kv_norm_w, kv_norm_b,
                 in_proj_w, in_proj_b, out_proj_w, out_proj_b):
    f32 = np.float32
    Wq, Wk, Wv = (np.asarray(in_proj_w[i * H:(i + 1) * H], f32) for i in range(3))
    bq, bk, bv = (np.asarray(in_proj_b[i * H:(i + 1) * H], f32) for i in range(3))
    kw = np.asarray(kv_norm_w, f32)
    kb = np.asarray(kv_norm_b, f32)

    # fold kv-norm affine into Wk/Wv (k/v = (z*kw + kb) @ W.T + b)
    WkT = (kw[:, None] * Wk.T.astype(f32))            # [h, o]
    WvT = (kw[:, None] * Wv.T.astype(f32))
    bk_f = bk + Wk.astype(f32) @ kb
    bv_f = bv + Wv.astype(f32) @ kb
    WqT = Wq.T.astype(f32).copy()                     # [h, o]

    def shuf_kc(WT):  # [h, o] -> [128, kc, o]
        return np.ascontiguousarray(WT.reshape(HC, 128, H).transpose(1, 0, 2))

    wq_s = shuf_kc(WqT)
    wv_s = shuf_kc(WvT)
    wk_s = np.ascontiguousarray(
        WkT.reshape(HC, 128, NP, 128).transpose(1, 2, 0, 3))
    WoT = out_proj_w.T.astype(f32)                    # [ho, o]
    wo_s = np.ascontiguousarray(WoT.reshape(NH, 64, H).transpose(1, 0, 2))

    def to_bf16(x):
        import jax.numpy as jnp
        return np.asarray(jnp.asarray(x, dtype=jnp.bfloat16))

    bkt = np.ascontiguousarray(bk_f.reshape(HC, 128).T)  # [128, kc]

    common = {
        "wq": to_bf16(wq_s), "wv": to_bf16(wv_s), "wk": to_bf16(wk_s),
        "wo": to_bf16(wo_s),
        "bq": (bq * SCALE).astype(f32), "bkt": bkt, "bv": bv_f.astype(f32),
        "bo": np.asarray(out_proj_b, f32),
        "wqn": np.asarray(q_norm_w, f32), "bqn": np.asarray(q_norm_b, f32),
        "ident_f32": np.eye(128, dtype=f32),
        "ident_bf16": to_bf16(np.eye(128, dtype=f32)),
    }
    in_maps = []
    for b in range(B):
        qm = np.asarray(desc_attention_mask[b], f32)
        den = max(float(qm.sum()), 1e-8)
        m = dict(common)
        m["desc"] = np.ascontiguousarray(np.asarray(desc_tokens[b], f32))
        m["dna"] = np.ascontiguousarray(np.asarray(dna_tokens[b], f32))
        m["kmask"] = to_bf16(np.asarray(dna_attention_mask[b], f32))
        m["qmask"] = qm[:, None].copy()
        m["qmask_pool"] = (qm / den)[:, None].astype(f32)
        in_maps.append(m)
    return in_maps


def kernel(**inputs):
    if "nc" not in _CACHE:
        _CACHE["nc"] = _build_program()
    nc = _CACHE["nc"]

    onw = np.asarray(inputs["out_norm_w"], np.float32)
    onb = np.asarray(inputs["out_norm_b"], np.float32)
    in_maps = _prep_inputs(
        inputs["dna_tokens"], inputs["desc_tokens"],
        inputs["dna_attention_mask"], inputs["desc_attention_mask"],
        inputs["q_norm_w"], inputs["q_norm_b"],
        inputs["kv_norm_w"], inputs["kv_norm_b"],
        inputs["in_proj_w"], inputs["in_proj_b"],
        inputs["out_proj_w"], inputs["out_proj_b"])

    res = None
    for attempt in range(3):
        try:
            res = run_bass_kernel_spmd(nc, in_maps, list(range(B)))
            break
        except Exception:
            if attempt == 2:
                raise
    assert res is not None
    pooled = np.stack([res.results[b]["pooled"][0] for b in range(B)])
    pooled = pooled * onw[None, :] + onb[None, :]
    attn = np.stack([res.results[b]["attn"] for b in range(B)])
    return pooled.astype(np.float32), attn.astype(np.float32)
